# revision 6
# baseline (speedup 1.0000x reference)
"""Multi-head self-attention (B=4, S=2048, D=1024, H=16, Hd=64) on 8 TRN2 cores.

Sharding: tensor-parallel over heads. Core c owns heads 2c, 2c+1:
  - Wq/Wk/Wv column slices [:, 128c:128(c+1)], biases likewise
  - computes Q^T/K^T [128hd, 8192tok] (f32r) and V (bf16) for its 2 heads
  - flash-style attention per (batch, head): S^T = K^T.T @ Q^T tiles,
    exp on ACT (scale=1/8 folded in), AV + denominator via ones-append,
    reciprocal + PE ones-broadcast, normalize on DVE -> A^T (bf16)
  - 2 AllGathers (one per local head row-block) assemble A'^T [512, 8192] x2
  - out-proj: out^T[:, c-slice] = wo_perm_c.T @ A'^T + bo_c  (bf16 matmul,
    fp32 psum), host transposes/concats column slices.

Matmul dtype: float32r (~1.5e-4 rel err, full PE rate at N>=512) for
projections/scores; bf16 for the P/V/AV/out-proj path (psum always fp32).
"""
import numpy as np

B, S, D, H, HD = 4, 2048, 1024, 16, 64
N_CORES = 8
TOK = B * S            # 8192
HPC = H // N_CORES     # 2 heads per core
CW = HPC * HD          # 128 cols per core
QB = 512               # query block
NKT = S // 128         # 16 kt chunks per batch
NQB = S // QB          # 4 q blocks per batch
NTB = TOK // 512       # 16 token blocks overall

_CACHE = {}


def _build():
    import concourse.bacc as bacc
    import concourse.mybir as mybir
    import concourse.tile as tile

    F32 = mybir.dt.float32
    F32R = mybir.dt.float32r
    BF16 = mybir.dt.bfloat16
    AF = mybir.ActivationFunctionType

    nc = bacc.Bacc(trn_type="TRN2", target_bir_lowering=False, debug=False,
                   num_devices=N_CORES)

    xT = nc.dram_tensor("xT", [D, TOK], F32, kind="ExternalInput")
    wq = nc.dram_tensor("wq", [D, CW], F32, kind="ExternalInput")
    wk = nc.dram_tensor("wk", [D, CW], F32, kind="ExternalInput")
    wv = nc.dram_tensor("wv", [D, CW], F32, kind="ExternalInput")
    wo = nc.dram_tensor("wo", [D, CW], F32, kind="ExternalInput")  # row-permuted
    bq = nc.dram_tensor("bq", [CW, 1], F32, kind="ExternalInput")
    bk = nc.dram_tensor("bk", [CW, 1], F32, kind="ExternalInput")
    bv = nc.dram_tensor("bv", [CW, 1], F32, kind="ExternalInput")
    bo = nc.dram_tensor("bo", [CW, 1], F32, kind="ExternalInput")
    outT = nc.dram_tensor("outT", [CW, TOK], F32, kind="ExternalOutput")

    with tile.TileContext(nc) as tc:
        with tc.tile_pool(name="sb", bufs=1) as sb, \
             tc.tile_pool(name="dram", bufs=1, space="DRAM") as dram:
            # ---------------- prologue: weights, biases, constants ----------
            # weight chunk layout: [128 dims, 8 chunks * 128 cols]
            wst = sb.tile([128, D], F32, tag="wst", bufs=2, name="wst0")
            nc.sync.dma_start(wst[:].rearrange("p (k m) -> p k m", k=8), wq.ap().rearrange("(k p) m -> p k m", p=128))
            wq_r = sb.tile([128, D], F32R, tag="wq_r", name="wq_r")
            nc.vector.tensor_copy(wq_r[:], wst[:])

            wst2 = sb.tile([128, D], F32, tag="wst", bufs=2, name="wst1")
            nc.sync.dma_start(wst2[:].rearrange("p (k m) -> p k m", k=8), wk.ap().rearrange("(k p) m -> p k m", p=128))
            wk_r = sb.tile([128, D], F32R, tag="wk_r", name="wk_r")
            nc.vector.tensor_copy(wk_r[:], wst2[:])

            wst3 = sb.tile([128, D], F32, tag="wst", bufs=2, name="wst2")
            nc.sync.dma_start(wst3[:].rearrange("p (k m) -> p k m", k=8), wv.ap().rearrange("(k p) m -> p k m", p=128))
            wv_r = sb.tile([128, D], F32R, tag="wv_r", name="wv_r")
            nc.vector.tensor_copy(wv_r[:], wst3[:])

            wst4 = sb.tile([128, D], F32, tag="wst", bufs=2, name="wst3")
            nc.sync.dma_start(wst4[:].rearrange("p (k m) -> p k m", k=8), wo.ap().rearrange("(k p) m -> p k m", p=128))
            wo_b = sb.tile([128, D], BF16, tag="wo_b", name="wo_b")
            nc.vector.tensor_copy(wo_b[:], wst4[:])

            bq_t = sb.tile([CW, 1], F32, tag="bq_t", name="bq_t")
            nc.sync.dma_start(bq_t[:], bq[:])
            bk_t = sb.tile([CW, 1], F32, tag="bk_t", name="bk_t")
            nc.sync.dma_start(bk_t[:], bk[:])
            bv_t = sb.tile([CW, 1], F32, tag="bv_t", name="bv_t")
            nc.sync.dma_start(bv_t[:], bv[:])
            bo_t = sb.tile([CW, 1], F32, tag="bo_t", name="bo_t")
            nc.sync.dma_start(bo_t[:], bo[:])

            ones_f = sb.tile([65, 64], F32, tag="ones_f", name="ones_f")
            nc.vector.memset(ones_f[:], 1.0)
            ones_r = sb.tile([65, 64], F32R, tag="ones_r", name="ones_r")
            nc.vector.tensor_copy(ones_r[:], ones_f[:])

            # persistent activations
            qt_sb, kt_sb, vt_sb = {}, {}, {}
            at_sb = [sb.tile([64, TOK], BF16, tag=f"at{h}", name=f"at{h}")
                     for h in range(2)]

            agin = [dram.tile([64, TOK], BF16, tag=f"agin{h}", name=f"agin{h}")
                    for h in range(2)]
            agout = [dram.tile([64 * N_CORES, TOK], BF16, tag=f"agout{h}",
                               addr_space="Shared", name=f"agout{h}")
                     for h in range(2)]

            with tc.tile_pool(name="ps12", bufs=1, space="PSUM") as ps:
                for b in range(B):
                    t0 = b * S
                    # ---------------- P1: projections for batch b ----------
                    qt = sb.tile([128, S], F32R, tag="qt_sb", bufs=2,
                                 name=f"qt{b}")
                    kt = sb.tile([128, S], F32R, tag="kt_sb", bufs=2,
                                 name=f"kt{b}")
                    vt = sb.tile([128, S], BF16, tag="vt_sb", bufs=2,
                                 name=f"vt{b}")
                    qt_sb[b], kt_sb[b], vt_sb[b] = qt, kt, vt
                    for tb in range(S // 512):
                        g0 = t0 + tb * 512
                        xr = []
                        for k in range(8):
                            xs = sb.tile([128, 512], F32, tag="xstage", bufs=3,
                                         name=f"xs{b}_{tb}_{k}")
                            nc.sync.dma_start(
                                xs[:], xT[k * 128:(k + 1) * 128, g0:g0 + 512])
                            xk = sb.tile([128, 512], F32R, tag="xr", bufs=9,
                                         name=f"xr{b}_{tb}_{k}")
                            nc.vector.tensor_copy(xk[:], xs[:])
                            xr.append(xk)
                        for w_r, out_sb, bias, odt in (
                                (wq_r, qt, bq_t, None),
                                (wk_r, kt, bk_t, None),
                                (wv_r, vt, bv_t, None)):
                            pp = ps.tile([128, 512], F32, tag="proj", bufs=2,
                                         name=f"pp{b}_{tb}")
                            for k in range(8):
                                nc.tensor.matmul(
                                    pp[:], w_r[:, k * 128:(k + 1) * 128],
                                    xr[k][:], start=(k == 0), stop=(k == 7))
                            nc.vector.tensor_scalar_add(
                                out_sb[:, tb * 512:(tb + 1) * 512], pp[:],
                                bias[:])
                    # V_ext tiles: [128 tok, 65] per (kt, head), bf16
                    vext = {}
                    for ktc in range(NKT):
                        for h in range(2):
                            ve = sb.tile([128, 65], BF16, tag="vext", bufs=36,
                                         name=f"ve{b}_{ktc}_{h}")
                            nc.sync.dma_start_transpose(
                                ve[:, 0:64],
                                vt[h * 64:(h + 1) * 64,
                                   ktc * 128:(ktc + 1) * 128])
                            nc.vector.memset(ve[:, 64:65], 1.0)
                            vext[(ktc, h)] = ve

                    # ---------------- P2: attention for batch b ------------
                    for qb in range(NQB):
                        q0 = qb * QB
                        pav = [ps.tile([65, QB], F32, tag=f"av{h}", bufs=1,
                                       name=f"pav{b}_{qb}_{h}")
                               for h in range(2)]
                        ptiles = {}
                        for ktc in range(NKT):
                            for h in range(2):
                                s_ps = ps.tile([128, QB], F32, tag=f"s{h}",
                                               bufs=1, name=f"s{b}{qb}{ktc}{h}")
                                nc.tensor.matmul(
                                    s_ps[:],
                                    kt[h * 64:(h + 1) * 64,
                                       ktc * 128:(ktc + 1) * 128],
                                    qt[h * 64:(h + 1) * 64, q0:q0 + QB],
                                    start=True, stop=True,
                                    tile_position=(h * 64, 0))
                                pt = sb.tile([128, QB], BF16, tag="p_sb",
                                             bufs=36, name=f"p{b}{qb}{ktc}{h}")
                                nc.scalar.activation(pt[:], s_ps[:], AF.Exp,
                                                     scale=0.125)
                                ptiles[(ktc, h)] = pt
                        for ktc in range(NKT):
                            for h in range(2):
                                nc.tensor.matmul(
                                    pav[h][:], vext[(ktc, h)][:],
                                    ptiles[(ktc, h)][:],
                                    start=(ktc == 0), stop=(ktc == NKT - 1))
                        for h in range(2):
                            rc = sb.tile([65, QB], F32R, tag=f"rc{h}", bufs=2,
                                         name=f"rc{b}_{qb}_{h}")
                            with nc.allow_low_precision(
                                    reason="f32r recip feeds f32r bcast mm"):
                                nc.vector.reciprocal(rc[64:65, :],
                                                     pav[h][64:65, :])
                            bc = ps.tile([64, QB], F32, tag=f"bc{h}", bufs=1,
                                         name=f"bc{b}_{qb}_{h}")
                            nc.tensor.matmul(bc[:], ones_r[64:65, 0:64],
                                             rc[64:65, :], start=True,
                                             stop=True, tile_position=(64, 0))
                            bcs = sb.tile([64, QB], F32, tag=f"bcs{h}", bufs=2,
                                          name=f"bcs{b}_{qb}_{h}")
                            nc.vector.tensor_copy(bcs[:], bc[:])
                            nc.vector.tensor_mul(
                                at_sb[h][:, t0 + q0:t0 + q0 + QB],
                                pav[h][0:64, :], bcs[:])
                    for h in range(2):
                        nc.sync.dma_start(agin[h][:, t0:t0 + S],
                                          at_sb[h][:, t0:t0 + S])

            # ---------------- AllGather ------------------------------------
            for h in range(2):
                nc.gpsimd.collective_compute(
                    "AllGather", mybir.AluOpType.bypass,
                    replica_groups=[list(range(N_CORES))],
                    ins=[agin[h][:]], outs=[agout[h][:]],
                )

            # ---------------- P3: out-projection ---------------------------
            with tc.tile_pool(name="ps3", bufs=1, space="PSUM") as ps3:
                for tb in range(NTB):
                    g0 = tb * 512
                    po = ps3.tile([128, 512], F32, tag="outp", bufs=2,
                                  name=f"po{tb}")
                    for kc in range(8):
                        ast = sb.tile([128, 512], BF16, tag="ast", bufs=10,
                                      name=f"ast{tb}_{kc}")
                        src = agout[kc // 4]
                        r0 = (kc % 4) * 128
                        nc.sync.dma_start(ast[:],
                                          src[r0:r0 + 128, g0:g0 + 512])
                        nc.tensor.matmul(po[:],
                                         wo_b[:, kc * 128:(kc + 1) * 128],
                                         ast[:], start=(kc == 0),
                                         stop=(kc == 7))
                    ot = sb.tile([128, 512], F32, tag="ot", bufs=3,
                                 name=f"ot{tb}")
                    nc.vector.tensor_scalar_add(ot[:], po[:], bo_t[:])
                    nc.sync.dma_start(outT[:, g0:g0 + 512], ot[:])

    nc.compile()
    return nc


def _get_nc():
    if "nc" not in _CACHE:
        _CACHE["nc"] = _build()
    return _CACHE["nc"]


def kernel(x, Wq, bq, Wk, bk, Wv, bv, Wo, bo):
    from concourse import bass_utils

    x = np.asarray(x, dtype=np.float32)
    Wq, Wk, Wv, Wo = (np.asarray(w, dtype=np.float32) for w in (Wq, Wk, Wv, Wo))
    bq, bk, bv, bo = (np.asarray(v, dtype=np.float32) for v in (bq, bk, bv, bo))

    xT = np.ascontiguousarray(x.reshape(TOK, D).T)
    # Wo rows permuted to match [all-h0-blocks; all-h1-blocks] AllGather order:
    # A'^T row r*64+t of gather h corresponds to head (2r+h), dim t -> Wo row
    # r*128 + h*64 + t.
    wo4 = Wo.reshape(N_CORES, 2, HD, D)
    wo_perm = np.concatenate([wo4[:, 0], wo4[:, 1]], axis=0).reshape(D, D)

    in_maps = []
    for c in range(N_CORES):
        cs = slice(c * CW, (c + 1) * CW)
        in_maps.append({
            "xT": xT,
            "wq": np.ascontiguousarray(Wq[:, cs]),
            "wk": np.ascontiguousarray(Wk[:, cs]),
            "wv": np.ascontiguousarray(Wv[:, cs]),
            "wo": np.ascontiguousarray(wo_perm[:, cs]),
            "bq": np.ascontiguousarray(bq[cs].reshape(CW, 1)),
            "bk": np.ascontiguousarray(bk[cs].reshape(CW, 1)),
            "bv": np.ascontiguousarray(bv[cs].reshape(CW, 1)),
            "bo": np.ascontiguousarray(bo[cs].reshape(CW, 1)),
        })

    nc = _get_nc()
    res = bass_utils.run_bass_kernel_spmd(nc, in_maps,
                                          core_ids=list(range(N_CORES)))
    _CACHE["last_results"] = res

    out = np.empty((TOK, D), dtype=np.float32)
    for c in range(N_CORES):
        out[:, c * CW:(c + 1) * CW] = res.results[c]["outT"].T
    return out.reshape(B, S, D)


# revision 8
# speedup vs baseline: 1.1821x; 1.1821x over previous
"""Multi-head self-attention (B=4, S=2048, D=1024, H=16, Hd=64) on 8 TRN2 cores.

Sharding: tensor-parallel over heads. Core c owns heads 2c, 2c+1:
  - computes Q^T/K^T [128hd, tok] (f32r) and V (bf16) for its 2 heads
  - flash-style attention per (batch, head): S^T = K^T.T @ Q^T into
    [128,1024] psum stripes, exp on ACT (scale=1/8 folded in), AV +
    denominator via ones-append (M=65), reciprocal_approx_fast +
    PE ones-broadcast, normalize on DVE -> A^T (bf16)
  - 8 pipelined AllGathers (one per batch x local-head row-block)
  - out-proj: out^T[:, c-slice] = wo_perm_c.T @ A'^T + bo_c (bf16 matmul,
    fp32 psum); host transposes/concats column slices.

Matmul dtype: float32r (~1.5e-4 rel err, full PE rate at N>=512) for
projections/scores; bf16 for the P/V/AV/out-proj path (psum always fp32).
"""
import numpy as np

B, S, D, H, HD = 4, 2048, 1024, 16, 64
N_CORES = 8
TOK = B * S            # 8192
HPC = H // N_CORES     # 2 heads per core
CW = HPC * HD          # 128 cols per core
QS = 1024              # query stripe
NKT = S // 128         # 16 kt chunks per batch
NQS = S // QS          # 2 q stripes per batch
NTB = TOK // 512       # 16 token blocks overall

_CACHE = {}


def _build():
    import concourse.bacc as bacc
    import concourse.mybir as mybir
    import concourse.tile as tile

    F32 = mybir.dt.float32
    F32R = mybir.dt.float32r
    BF16 = mybir.dt.bfloat16
    AF = mybir.ActivationFunctionType

    nc = bacc.Bacc(trn_type="TRN2", target_bir_lowering=False, debug=False,
                   num_devices=N_CORES)

    xT = nc.dram_tensor("xT", [D, TOK], F32, kind="ExternalInput")
    wq = nc.dram_tensor("wq", [D, CW], F32, kind="ExternalInput")
    wk = nc.dram_tensor("wk", [D, CW], F32, kind="ExternalInput")
    wv = nc.dram_tensor("wv", [D, CW], F32, kind="ExternalInput")
    wo = nc.dram_tensor("wo", [D, CW], F32, kind="ExternalInput")  # row-permuted
    bq = nc.dram_tensor("bq", [CW, 1], F32, kind="ExternalInput")
    bk = nc.dram_tensor("bk", [CW, 1], F32, kind="ExternalInput")
    bv = nc.dram_tensor("bv", [CW, 1], F32, kind="ExternalInput")
    bo = nc.dram_tensor("bo", [CW, 1], F32, kind="ExternalInput")
    outT = nc.dram_tensor("outT", [CW, TOK], F32, kind="ExternalOutput")

    with tile.TileContext(nc) as tc:
        with tc.tile_pool(name="sb", bufs=1) as sb, \
             tc.tile_pool(name="dram", bufs=1, space="DRAM") as dram:
            # ---------------- prologue: weights, biases, constants --------
            w_r = {}
            for wname, wdram, odt in (("wq", wq, F32R), ("wk", wk, F32R),
                                      ("wv", wv, F32R), ("wo", wo, BF16)):
                wst = sb.tile([128, D], F32, tag="wst", bufs=2,
                              name=f"wst_{wname}")
                nc.sync.dma_start(
                    wst[:].rearrange("p (k m) -> p k m", k=8),
                    wdram.ap().rearrange("(k p) m -> p k m", p=128))
                wr = sb.tile([128, D], odt, tag=f"{wname}_r",
                             name=f"{wname}_r")
                nc.vector.tensor_copy(wr[:], wst[:])
                w_r[wname] = wr
            wq_r, wk_r, wv_r, wo_b = w_r["wq"], w_r["wk"], w_r["wv"], w_r["wo"]

            bias_t = {}
            for bname, bdram in (("bq", bq), ("bk", bk), ("bv", bv),
                                 ("bo", bo)):
                bt_ = sb.tile([CW, 1], F32, tag=f"{bname}_t", name=f"{bname}_t")
                nc.sync.dma_start(bt_[:], bdram[:])
                bias_t[bname] = bt_

            ones_f = sb.tile([65, 64], F32, tag="ones_f", name="ones_f")
            nc.vector.memset(ones_f[:], 1.0)

            agin = {}
            agout = {}
            for b in range(B):
                for h in range(2):
                    agin[(b, h)] = dram.tile([64, S], BF16, tag=f"agi{b}{h}",
                                             name=f"agi{b}{h}")
                    agout[(b, h)] = dram.tile(
                        [64 * N_CORES, S], BF16, tag=f"ago{b}{h}",
                        addr_space="Shared", name=f"ago{b}{h}")

            with tc.tile_pool(name="ps12", bufs=1, space="PSUM") as ps:
                for b in range(B):
                    t0 = b * S
                    # ------------- P1: projections for batch b ------------
                    qt = sb.tile([128, S], F32R, tag="qt_sb", bufs=2,
                                 name=f"qt{b}")
                    kt = sb.tile([128, S], F32R, tag="kt_sb", bufs=2,
                                 name=f"kt{b}")
                    vt = sb.tile([128, S], BF16, tag="vt_sb", bufs=2,
                                 name=f"vt{b}")
                    for tb in range(S // 512):
                        g0 = t0 + tb * 512
                        xr = []
                        for k in range(8):
                            xs = sb.tile([128, 512], F32, tag="xstage", bufs=3,
                                         name=f"xs{b}_{tb}_{k}")
                            nc.sync.dma_start(
                                xs[:], xT[k * 128:(k + 1) * 128, g0:g0 + 512])
                            xk = sb.tile([128, 512], F32R, tag="xr", bufs=9,
                                         name=f"xr{b}_{tb}_{k}")
                            nc.vector.tensor_copy(xk[:], xs[:])
                            xr.append(xk)
                        for w_, out_sb, bias in ((wq_r, qt, bias_t["bq"]),
                                                 (wk_r, kt, bias_t["bk"]),
                                                 (wv_r, vt, bias_t["bv"])):
                            pp = ps.tile([128, 512], F32, tag="proj", bufs=2,
                                         name=f"pp{b}_{tb}")
                            for k in range(8):
                                nc.tensor.matmul(
                                    pp[:], w_[:, k * 128:(k + 1) * 128],
                                    xr[k][:], start=(k == 0), stop=(k == 7))
                            nc.vector.tensor_scalar_add(
                                out_sb[:, tb * 512:(tb + 1) * 512], pp[:],
                                bias[:])
                    # V_ext tiles [128 tok, 65] bf16 per (kt, head)
                    vext = {}
                    for ktc in range(NKT):
                        for h in range(2):
                            ve = sb.tile([128, 65], BF16, tag="vext", bufs=36,
                                         name=f"ve{b}_{ktc}_{h}")
                            nc.sync.dma_start_transpose(
                                ve[:, 0:64],
                                vt[h * 64:(h + 1) * 64,
                                   ktc * 128:(ktc + 1) * 128])
                            nc.vector.memset(ve[:, 64:65], 1.0)
                            vext[(ktc, h)] = ve

                    # ------------- P2: attention for batch b --------------
                    for h in range(2):
                        at_t = sb.tile([64, S], BF16, tag="at_t", bufs=3,
                                       name=f"at{b}_{h}")
                        for qs_i in range(NQS):
                            q0 = qs_i * QS
                            pav = ps.tile([65, QS], F32, tag="av", bufs=1,
                                          name=f"pav{b}_{h}_{qs_i}")
                            for ktc in range(NKT):
                                s_ps = ps.tile([128, QS], F32, tag="s",
                                               bufs=2,
                                               name=f"s{b}{h}{qs_i}{ktc}")
                                for half in range(2):
                                    nc.tensor.matmul(
                                        s_ps[:, half * 512:(half + 1) * 512],
                                        kt[h * 64:(h + 1) * 64,
                                           ktc * 128:(ktc + 1) * 128],
                                        qt[h * 64:(h + 1) * 64,
                                           q0 + half * 512:q0 + (half + 1) * 512],
                                        start=True, stop=True,
                                        tile_position=(h * 64, 0))
                                pt = sb.tile([128, QS], BF16, tag="p_sb",
                                             bufs=4, name=f"p{b}{h}{qs_i}{ktc}")
                                nc.scalar.activation(pt[:], s_ps[:], AF.Exp,
                                                     scale=0.125)
                                for half in range(2):
                                    nc.tensor.matmul(
                                        pav[:, half * 512:(half + 1) * 512],
                                        vext[(ktc, h)][:],
                                        pt[:, half * 512:(half + 1) * 512],
                                        start=(ktc == 0), stop=(ktc == NKT - 1))
                            # drain av psum to sbuf, normalize
                            araw = sb.tile([65, QS], F32, tag="araw", bufs=2,
                                           name=f"ar{b}_{h}_{qs_i}")
                            nc.vector.tensor_copy(araw[:], pav[:])
                            rcf = sb.tile([65, QS], F32, tag="rcf", bufs=2,
                                          name=f"rcf{b}_{h}_{qs_i}")
                            nc.vector.reciprocal_approx_fast(rcf[:],
                                                                 araw[:])
                            bc = ps.tile([64, QS], F32, tag="s", bufs=2,
                                         name=f"bc{b}_{h}_{qs_i}")
                            for half in range(2):
                                nc.tensor.matmul(
                                    bc[:, half * 512:(half + 1) * 512],
                                    ones_f[64:65, 0:64],
                                    rcf[64:65, half * 512:(half + 1) * 512],
                                    start=True, stop=True,
                                    tile_position=(64, 0))
                            bcs = sb.tile([64, QS], F32, tag="bcs", bufs=2,
                                          name=f"bcs{b}_{h}_{qs_i}")
                            nc.vector.tensor_copy(bcs[:], bc[:])
                            nc.vector.tensor_mul(at_t[:, q0:q0 + QS],
                                                 araw[0:64, :], bcs[:])
                        nc.sync.dma_start(agin[(b, h)][:], at_t[:])
                        nc.gpsimd.collective_compute(
                            "AllGather", mybir.AluOpType.bypass,
                            replica_groups=[list(range(N_CORES))],
                            ins=[agin[(b, h)][:]], outs=[agout[(b, h)][:]],
                        )

            # ---------------- P3: out-projection -------------------------
            with tc.tile_pool(name="ps3", bufs=1, space="PSUM") as ps3:
                for tb in range(NTB):
                    b = tb // 4
                    c0 = (tb % 4) * 512
                    po = ps3.tile([128, 512], F32, tag="outp", bufs=2,
                                  name=f"po{tb}")
                    for kc in range(8):
                        ast = sb.tile([128, 512], BF16, tag="ast", bufs=10,
                                      name=f"ast{tb}_{kc}")
                        src = agout[(b, kc // 4)]
                        r0 = (kc % 4) * 128
                        nc.sync.dma_start(ast[:],
                                          src[r0:r0 + 128, c0:c0 + 512])
                        nc.tensor.matmul(po[:],
                                         wo_b[:, kc * 128:(kc + 1) * 128],
                                         ast[:], start=(kc == 0),
                                         stop=(kc == 7))
                    ot = sb.tile([128, 512], F32, tag="ot", bufs=3,
                                 name=f"ot{tb}")
                    nc.vector.tensor_scalar_add(ot[:], po[:], bias_t["bo"][:])
                    nc.sync.dma_start(outT[:, tb * 512:(tb + 1) * 512], ot[:])

    nc.compile()
    return nc


def _get_nc():
    if "nc" not in _CACHE:
        _CACHE["nc"] = _build()
    return _CACHE["nc"]


def _make_in_maps(x, Wq, bq, Wk, bk, Wv, bv, Wo, bo):
    x = np.asarray(x, dtype=np.float32)
    Wq, Wk, Wv, Wo = (np.asarray(w, dtype=np.float32) for w in (Wq, Wk, Wv, Wo))
    bq, bk, bv, bo = (np.asarray(v, dtype=np.float32) for v in (bq, bk, bv, bo))

    xT = np.ascontiguousarray(x.reshape(TOK, D).T)
    # Wo rows permuted: gathered A'^T row r*64+t of head-block h corresponds
    # to head (2r+h), dim t -> original Wo row r*128 + h*64 + t.
    wo4 = Wo.reshape(N_CORES, 2, HD, D)
    wo_perm = np.concatenate([wo4[:, 0], wo4[:, 1]], axis=0).reshape(D, D)

    in_maps = []
    for c in range(N_CORES):
        cs = slice(c * CW, (c + 1) * CW)
        in_maps.append({
            "xT": xT,
            "wq": np.ascontiguousarray(Wq[:, cs]),
            "wk": np.ascontiguousarray(Wk[:, cs]),
            "wv": np.ascontiguousarray(Wv[:, cs]),
            "wo": np.ascontiguousarray(wo_perm[:, cs]),
            "bq": np.ascontiguousarray(bq[cs].reshape(CW, 1)),
            "bk": np.ascontiguousarray(bk[cs].reshape(CW, 1)),
            "bv": np.ascontiguousarray(bv[cs].reshape(CW, 1)),
            "bo": np.ascontiguousarray(bo[cs].reshape(CW, 1)),
        })
    return in_maps


def kernel(x, Wq, bq, Wk, bk, Wv, bv, Wo, bo):
    from concourse import bass_utils

    in_maps = _make_in_maps(x, Wq, bq, Wk, bk, Wv, bv, Wo, bo)
    nc = _get_nc()
    res = bass_utils.run_bass_kernel_spmd(nc, in_maps,
                                          core_ids=list(range(N_CORES)))
    _CACHE["last_results"] = res

    out = np.empty((TOK, D), dtype=np.float32)
    for c in range(N_CORES):
        out[:, c * CW:(c + 1) * CW] = res.results[c]["outT"].T
    return out.reshape(B, S, D)


# revision 11
# speedup vs baseline: 1.2765x; 1.0799x over previous
"""Multi-head self-attention (B=4, S=2048, D=1024, H=16, Hd=64) on 8 TRN2 cores.

Sharding: tensor-parallel over heads. Core c owns heads 2c, 2c+1:
  - computes Q^T/K^T [128hd, tok] (f32r) and V (bf16) for its 2 heads
  - flash-style attention per (batch, head): S^T = K^T.T @ Q^T into
    [128,1024] psum stripes, exp on ACT (scale=1/8 folded in), AV +
    denominator via ones-append (M=65), reciprocal_approx_fast +
    PE ones-broadcast, normalize on DVE -> A^T (bf16)
  - 8 pipelined AllGathers (one per batch x local-head row-block)
  - out-proj: out^T[:, c-slice] = wo_perm_c.T @ A'^T + bo_c (bf16 matmul,
    fp32 psum); host transposes/concats column slices.

Matmul dtype: float32r (~1.5e-4 rel err, full PE rate at N>=512) for
projections/scores; bf16 for the P/V/AV/out-proj path (psum always fp32).
"""
import numpy as np

B, S, D, H, HD = 4, 2048, 1024, 16, 64
N_CORES = 8
TOK = B * S            # 8192
HPC = H // N_CORES     # 2 heads per core
CW = HPC * HD          # 128 cols per core
QS = 1024              # query stripe
NKT = S // 128         # 16 kt chunks per batch
NQS = S // QS          # 2 q stripes per batch
NTB = TOK // 512       # 16 token blocks overall

_CACHE = {}


def _build():
    import concourse.bacc as bacc
    import concourse.mybir as mybir
    import concourse.tile as tile

    F32 = mybir.dt.float32
    F32R = mybir.dt.float32r
    BF16 = mybir.dt.bfloat16
    AF = mybir.ActivationFunctionType

    nc = bacc.Bacc(trn_type="TRN2", target_bir_lowering=False, debug=False,
                   num_devices=N_CORES)

    xT = nc.dram_tensor("xT", [D, TOK], F32, kind="ExternalInput")
    wq = nc.dram_tensor("wq", [D, CW], F32, kind="ExternalInput")
    wk = nc.dram_tensor("wk", [D, CW], F32, kind="ExternalInput")
    wv = nc.dram_tensor("wv", [D, CW], F32, kind="ExternalInput")
    wo = nc.dram_tensor("wo", [D, CW], F32, kind="ExternalInput")  # row-permuted
    bq = nc.dram_tensor("bq", [CW, 1], F32, kind="ExternalInput")
    bk = nc.dram_tensor("bk", [CW, 1], F32, kind="ExternalInput")
    bv = nc.dram_tensor("bv", [CW, 1], F32, kind="ExternalInput")
    bo = nc.dram_tensor("bo", [CW, 1], F32, kind="ExternalInput")
    outT = nc.dram_tensor("outT", [CW, TOK], F32, kind="ExternalOutput")

    with tile.TileContext(nc) as tc:
        with tc.tile_pool(name="sb", bufs=1) as sb, \
             tc.tile_pool(name="dram", bufs=1, space="DRAM") as dram:
            # ---------------- prologue: weights, biases, constants --------
            w_r = {}
            for wname, wdram, odt in (("wq", wq, F32R), ("wk", wk, F32R),
                                      ("wv", wv, F32R), ("wo", wo, BF16)):
                wst = sb.tile([128, D], F32, tag="wst", bufs=2,
                              name=f"wst_{wname}")
                nc.sync.dma_start(
                    wst[:].rearrange("p (k m) -> p k m", k=8),
                    wdram.ap().rearrange("(k p) m -> p k m", p=128))
                wr = sb.tile([128, D], odt, tag=f"{wname}_r",
                             name=f"{wname}_r")
                nc.vector.tensor_copy(wr[:], wst[:])
                w_r[wname] = wr
            wq_r, wk_r, wv_r, wo_b = w_r["wq"], w_r["wk"], w_r["wv"], w_r["wo"]

            bias_t = {}
            for bname, bdram in (("bq", bq), ("bk", bk), ("bv", bv),
                                 ("bo", bo)):
                bt_ = sb.tile([CW, 1], F32, tag=f"{bname}_t", name=f"{bname}_t")
                nc.sync.dma_start(bt_[:], bdram[:])
                bias_t[bname] = bt_

            agin = {}
            agout = {}
            for b in range(B):
                for h in range(2):
                    agin[(b, h)] = dram.tile([64, S], BF16, tag=f"agi{b}{h}",
                                             name=f"agi{b}{h}")
                    agout[(b, h)] = dram.tile(
                        [64 * N_CORES, S], BF16, tag=f"ago{b}{h}",
                        addr_space="Shared", name=f"ago{b}{h}")

            with tc.tile_pool(name="ps12", bufs=1, space="PSUM") as ps:
                for b in range(B):
                    t0 = b * S
                    # ------------- P1: projections for batch b ------------
                    qt = sb.tile([128, S], F32R, tag="qt_sb", bufs=2,
                                 name=f"qt{b}")
                    kt = sb.tile([128, S], F32R, tag="kt_sb", bufs=2,
                                 name=f"kt{b}")
                    vt = sb.tile([128, S], BF16, tag="vt_sb", bufs=2,
                                 name=f"vt{b}")
                    for tb in range(S // 512):
                        g0 = t0 + tb * 512
                        xr = []
                        for k in range(8):
                            xs = sb.tile([128, 512], F32, tag="xstage", bufs=3,
                                         name=f"xs{b}_{tb}_{k}")
                            nc.sync.dma_start(
                                xs[:], xT[k * 128:(k + 1) * 128, g0:g0 + 512])
                            xk = sb.tile([128, 512], F32R, tag="xr", bufs=9,
                                         name=f"xr{b}_{tb}_{k}")
                            nc.vector.tensor_copy(xk[:], xs[:])
                            xr.append(xk)
                        for w_, out_sb, bias in ((wq_r, qt, bias_t["bq"]),
                                                 (wk_r, kt, bias_t["bk"]),
                                                 (wv_r, vt, bias_t["bv"])):
                            pp = ps.tile([128, 512], F32, tag="proj", bufs=2,
                                         name=f"pp{b}_{tb}")
                            for k in range(8):
                                nc.tensor.matmul(
                                    pp[:], w_[:, k * 128:(k + 1) * 128],
                                    xr[k][:], start=(k == 0), stop=(k == 7))
                            nc.vector.tensor_scalar_add(
                                out_sb[:, tb * 512:(tb + 1) * 512], pp[:],
                                bias[:])
                    # V_ext tiles [128 tok, 65] bf16 per (kt, head)
                    vext = {}
                    for ktc in range(NKT):
                        for h in range(2):
                            ve = sb.tile([128, 128], BF16, tag="vext", bufs=36,
                                         name=f"ve{b}_{ktc}_{h}")
                            nc.vector.memset(ve[:, 0:64], 1.0)
                            nc.sync.dma_start_transpose(
                                ve[:, 64:128],
                                vt[h * 64:(h + 1) * 64,
                                   ktc * 128:(ktc + 1) * 128])
                            vext[(ktc, h)] = ve

                    # ------------- P2: attention for batch b --------------
                    for h in range(2):
                        at_t = sb.tile([128, S], BF16, tag="at_t", bufs=3,
                                       name=f"at{b}_{h}")
                        for qs_i in range(NQS):
                            q0 = qs_i * QS
                            pav = ps.tile([128, QS], F32, tag="av", bufs=1,
                                          name=f"pav{b}_{h}_{qs_i}")
                            for ktc in range(NKT):
                                s_ps = ps.tile([128, QS], F32, tag="s",
                                               bufs=2,
                                               name=f"s{b}{h}{qs_i}{ktc}")
                                for half in range(2):
                                    nc.tensor.matmul(
                                        s_ps[:, half * 512:(half + 1) * 512],
                                        kt[h * 64:(h + 1) * 64,
                                           ktc * 128:(ktc + 1) * 128],
                                        qt[h * 64:(h + 1) * 64,
                                           q0 + half * 512:q0 + (half + 1) * 512],
                                        start=True, stop=True,
                                        tile_position=(h * 64, 0))
                                pt = sb.tile([128, QS], BF16, tag="p_sb",
                                             bufs=4, name=f"p{b}{h}{qs_i}{ktc}")
                                nc.scalar.activation(pt[:], s_ps[:], AF.Exp,
                                                     scale=0.125)
                                for half in range(2):
                                    nc.tensor.matmul(
                                        pav[:, half * 512:(half + 1) * 512],
                                        vext[(ktc, h)][:],
                                        pt[:, half * 512:(half + 1) * 512],
                                        start=(ktc == 0), stop=(ktc == NKT - 1))
                            # drain av psum to sbuf, normalize
                            araw = sb.tile([128, QS], F32, tag="araw", bufs=2,
                                           name=f"ar{b}_{h}_{qs_i}")
                            nc.vector.tensor_copy(araw[:], pav[:])
                            rcf = sb.tile([128, QS], F32, tag="rcf", bufs=2,
                                          name=f"rcf{b}_{h}_{qs_i}")
                            nc.vector.reciprocal_approx_fast(rcf[:],
                                                                 araw[:])
                            bcs = sb.tile([128, QS], F32, tag="bcs", bufs=2,
                                          name=f"bcs{b}_{h}_{qs_i}")
                            nc.gpsimd.partition_broadcast(bcs[:],
                                                          rcf[0:1, :])
                            nc.vector.tensor_mul(at_t[64:128, q0:q0 + QS],
                                                 araw[64:128, :],
                                                 bcs[64:128, :])
                        nc.sync.dma_start(agin[(b, h)][:], at_t[64:128, :])
                        nc.gpsimd.collective_compute(
                            "AllGather", mybir.AluOpType.bypass,
                            replica_groups=[list(range(N_CORES))],
                            ins=[agin[(b, h)][:]], outs=[agout[(b, h)][:]],
                        )

            # ---------------- P3: out-projection -------------------------
            with tc.tile_pool(name="ps3", bufs=1, space="PSUM") as ps3:
                for tb in range(NTB):
                    b = tb // 4
                    c0 = (tb % 4) * 512
                    po = ps3.tile([128, 512], F32, tag="outp", bufs=2,
                                  name=f"po{tb}")
                    for kc in range(8):
                        ast = sb.tile([128, 512], BF16, tag="ast", bufs=10,
                                      name=f"ast{tb}_{kc}")
                        src = agout[(b, kc // 4)]
                        r0 = (kc % 4) * 128
                        nc.sync.dma_start(ast[:],
                                          src[r0:r0 + 128, c0:c0 + 512])
                        nc.tensor.matmul(po[:],
                                         wo_b[:, kc * 128:(kc + 1) * 128],
                                         ast[:], start=(kc == 0),
                                         stop=(kc == 7))
                    ot = sb.tile([128, 512], F32, tag="ot", bufs=3,
                                 name=f"ot{tb}")
                    nc.vector.tensor_scalar_add(ot[:], po[:], bias_t["bo"][:])
                    nc.sync.dma_start(outT[:, tb * 512:(tb + 1) * 512], ot[:])

    nc.compile()
    return nc


def _get_nc():
    if "nc" not in _CACHE:
        _CACHE["nc"] = _build()
    return _CACHE["nc"]


def _make_in_maps(x, Wq, bq, Wk, bk, Wv, bv, Wo, bo):
    x = np.asarray(x, dtype=np.float32)
    Wq, Wk, Wv, Wo = (np.asarray(w, dtype=np.float32) for w in (Wq, Wk, Wv, Wo))
    bq, bk, bv, bo = (np.asarray(v, dtype=np.float32) for v in (bq, bk, bv, bo))

    xT = np.ascontiguousarray(x.reshape(TOK, D).T)
    # Wo rows permuted: gathered A'^T row r*64+t of head-block h corresponds
    # to head (2r+h), dim t -> original Wo row r*128 + h*64 + t.
    wo4 = Wo.reshape(N_CORES, 2, HD, D)
    wo_perm = np.concatenate([wo4[:, 0], wo4[:, 1]], axis=0).reshape(D, D)

    in_maps = []
    for c in range(N_CORES):
        cs = slice(c * CW, (c + 1) * CW)
        in_maps.append({
            "xT": xT,
            "wq": np.ascontiguousarray(Wq[:, cs]),
            "wk": np.ascontiguousarray(Wk[:, cs]),
            "wv": np.ascontiguousarray(Wv[:, cs]),
            "wo": np.ascontiguousarray(wo_perm[:, cs]),
            "bq": np.ascontiguousarray(bq[cs].reshape(CW, 1)),
            "bk": np.ascontiguousarray(bk[cs].reshape(CW, 1)),
            "bv": np.ascontiguousarray(bv[cs].reshape(CW, 1)),
            "bo": np.ascontiguousarray(bo[cs].reshape(CW, 1)),
        })
    return in_maps


def kernel(x, Wq, bq, Wk, bk, Wv, bv, Wo, bo):
    from concourse import bass_utils

    in_maps = _make_in_maps(x, Wq, bq, Wk, bk, Wv, bv, Wo, bo)
    nc = _get_nc()
    res = bass_utils.run_bass_kernel_spmd(nc, in_maps,
                                          core_ids=list(range(N_CORES)))
    _CACHE["last_results"] = res

    out = np.empty((TOK, D), dtype=np.float32)
    for c in range(N_CORES):
        out[:, c * CW:(c + 1) * CW] = res.results[c]["outT"].T
    return out.reshape(B, S, D)


# revision 12
# speedup vs baseline: 1.2938x; 1.0135x over previous
"""Multi-head self-attention (B=4, S=2048, D=1024, H=16, Hd=64) on 8 TRN2 cores.

Sharding: tensor-parallel over heads. Core c owns heads 2c, 2c+1:
  - computes Q^T/K^T [128hd, tok] (f32r) and V (bf16) for its 2 heads
  - flash-style attention per (batch, head): S^T = K^T.T @ Q^T into
    [128,1024] psum stripes, exp on ACT (scale=1/8 folded in), AV +
    denominator via ones-append (M=65), reciprocal_approx_fast +
    PE ones-broadcast, normalize on DVE -> A^T (bf16)
  - 8 pipelined AllGathers (one per batch x local-head row-block)
  - out-proj: out^T[:, c-slice] = wo_perm_c.T @ A'^T + bo_c (bf16 matmul,
    fp32 psum); host transposes/concats column slices.

Matmul dtype: float32r (~1.5e-4 rel err, full PE rate at N>=512) for
projections/scores; bf16 for the P/V/AV/out-proj path (psum always fp32).
"""
import numpy as np

B, S, D, H, HD = 4, 2048, 1024, 16, 64
N_CORES = 8
TOK = B * S            # 8192
HPC = H // N_CORES     # 2 heads per core
CW = HPC * HD          # 128 cols per core
QS = 1024              # query stripe
NKT = S // 128         # 16 kt chunks per batch
NQS = S // QS          # 2 q stripes per batch
NTB = TOK // 512       # 16 token blocks overall

_CACHE = {}


def _build():
    import concourse.bacc as bacc
    import concourse.mybir as mybir
    import concourse.tile as tile

    F32 = mybir.dt.float32
    F32R = mybir.dt.float32r
    BF16 = mybir.dt.bfloat16
    AF = mybir.ActivationFunctionType

    nc = bacc.Bacc(trn_type="TRN2", target_bir_lowering=False, debug=False,
                   num_devices=N_CORES)

    xT = nc.dram_tensor("xT", [D, TOK], F32, kind="ExternalInput")
    wq = nc.dram_tensor("wq", [D, CW], F32, kind="ExternalInput")
    wk = nc.dram_tensor("wk", [D, CW], F32, kind="ExternalInput")
    wv = nc.dram_tensor("wv", [D, CW], F32, kind="ExternalInput")
    wo = nc.dram_tensor("wo", [D, CW], F32, kind="ExternalInput")  # row-permuted
    bq = nc.dram_tensor("bq", [CW, 1], F32, kind="ExternalInput")
    bk = nc.dram_tensor("bk", [CW, 1], F32, kind="ExternalInput")
    bv = nc.dram_tensor("bv", [CW, 1], F32, kind="ExternalInput")
    bo = nc.dram_tensor("bo", [CW, 1], F32, kind="ExternalInput")
    outT = nc.dram_tensor("outT", [CW, TOK], F32, kind="ExternalOutput")

    with tile.TileContext(nc) as tc:
        with tc.tile_pool(name="sb", bufs=1) as sb, \
             tc.tile_pool(name="dram", bufs=1, space="DRAM") as dram:
            # ---------------- prologue: weights, biases, constants --------
            w_r = {}
            for wname, wdram, odt in (("wq", wq, F32R), ("wk", wk, F32R),
                                      ("wv", wv, F32R), ("wo", wo, BF16)):
                wst = sb.tile([128, D], F32, tag="wst", bufs=2,
                              name=f"wst_{wname}")
                nc.sync.dma_start(
                    wst[:].rearrange("p (k m) -> p k m", k=8),
                    wdram.ap().rearrange("(k p) m -> p k m", p=128))
                wr = sb.tile([128, D], odt, tag=f"{wname}_r",
                             name=f"{wname}_r")
                nc.vector.tensor_copy(wr[:], wst[:])
                w_r[wname] = wr
            wq_r, wk_r, wv_r, wo_b = w_r["wq"], w_r["wk"], w_r["wv"], w_r["wo"]

            bias_t = {}
            for bname, bdram in (("bq", bq), ("bk", bk), ("bv", bv),
                                 ("bo", bo)):
                bt_ = sb.tile([CW, 1], F32, tag=f"{bname}_t", name=f"{bname}_t")
                nc.sync.dma_start(bt_[:], bdram[:])
                bias_t[bname] = bt_

            agin = {}
            agout = {}
            for b in range(B):
                for h in range(2):
                    agin[(b, h)] = dram.tile([64, S], BF16, tag=f"agi{b}{h}",
                                             name=f"agi{b}{h}")
                    agout[(b, h)] = dram.tile(
                        [64 * N_CORES, S], BF16, tag=f"ago{b}{h}",
                        addr_space="Shared", name=f"ago{b}{h}")

            with tc.tile_pool(name="ps12", bufs=1, space="PSUM") as ps:
                qkv = {}

                def emit_p1_tb(b, tb):
                    if tb == 0:
                        qkv[b] = (
                            sb.tile([128, S], F32R, tag="qt_sb", bufs=2,
                                    name=f"qt{b}"),
                            sb.tile([128, S], F32R, tag="kt_sb", bufs=2,
                                    name=f"kt{b}"),
                            sb.tile([128, S], BF16, tag="vt_sb", bufs=2,
                                    name=f"vt{b}"),
                        )
                    qt, kt, vt = qkv[b]
                    g0 = b * S + tb * 512
                    xr = []
                    for k in range(8):
                        xs = sb.tile([128, 512], F32, tag="xstage", bufs=3,
                                     name=f"xs{b}_{tb}_{k}")
                        nc.sync.dma_start(
                            xs[:], xT[k * 128:(k + 1) * 128, g0:g0 + 512])
                        xk = sb.tile([128, 512], F32R, tag="xr", bufs=9,
                                     name=f"xr{b}_{tb}_{k}")
                        nc.vector.tensor_copy(xk[:], xs[:])
                        xr.append(xk)
                    for w_, out_sb, bias in ((wq_r, qt, bias_t["bq"]),
                                             (wk_r, kt, bias_t["bk"]),
                                             (wv_r, vt, bias_t["bv"])):
                        pp = ps.tile([128, 512], F32, tag="proj", bufs=2,
                                     name=f"pp{b}_{tb}")
                        for k in range(8):
                            nc.tensor.matmul(
                                pp[:], w_[:, k * 128:(k + 1) * 128],
                                xr[k][:], start=(k == 0), stop=(k == 7))
                        nc.vector.tensor_scalar_add(
                            out_sb[:, tb * 512:(tb + 1) * 512], pp[:],
                            bias[:])

                vext = {}

                def emit_vext(b):
                    vt = qkv[b][2]
                    for ktc in range(NKT):
                        for h in range(2):
                            ve = sb.tile([128, 128], BF16, tag="vext",
                                         bufs=36, name=f"ve{b}_{ktc}_{h}")
                            nc.vector.memset(ve[:, 0:64], 1.0)
                            nc.sync.dma_start_transpose(
                                ve[:, 64:128],
                                vt[h * 64:(h + 1) * 64,
                                   ktc * 128:(ktc + 1) * 128])
                            vext[(b, ktc, h)] = ve

                at_tiles = {}

                def emit_p2_stripe(b, h, qs_i):
                    qt, kt, vt = qkv[b]
                    if qs_i == 0:
                        at_tiles[(b, h)] = sb.tile(
                            [128, S], BF16, tag="at_t", bufs=3,
                            name=f"at{b}_{h}")
                    at_t = at_tiles[(b, h)]
                    q0 = qs_i * QS
                    pav = ps.tile([128, QS], F32, tag="av", bufs=1,
                                  name=f"pav{b}_{h}_{qs_i}")
                    for ktc in range(NKT):
                        s_ps = ps.tile([128, QS], F32, tag="s", bufs=2,
                                       name=f"s{b}{h}{qs_i}{ktc}")
                        for half in range(2):
                            nc.tensor.matmul(
                                s_ps[:, half * 512:(half + 1) * 512],
                                kt[h * 64:(h + 1) * 64,
                                   ktc * 128:(ktc + 1) * 128],
                                qt[h * 64:(h + 1) * 64,
                                   q0 + half * 512:q0 + (half + 1) * 512],
                                start=True, stop=True,
                                tile_position=(h * 64, 0))
                        pt = sb.tile([128, QS], BF16, tag="p_sb",
                                     bufs=4, name=f"p{b}{h}{qs_i}{ktc}")
                        nc.scalar.activation(pt[:], s_ps[:], AF.Exp,
                                             scale=0.125)
                        for half in range(2):
                            nc.tensor.matmul(
                                pav[:, half * 512:(half + 1) * 512],
                                vext[(b, ktc, h)][:],
                                pt[:, half * 512:(half + 1) * 512],
                                start=(ktc == 0), stop=(ktc == NKT - 1))
                    araw = sb.tile([128, QS], F32, tag="araw", bufs=2,
                                   name=f"ar{b}_{h}_{qs_i}")
                    nc.vector.tensor_copy(araw[:], pav[:])
                    rcf = sb.tile([128, QS], F32, tag="rcf", bufs=2,
                                  name=f"rcf{b}_{h}_{qs_i}")
                    nc.vector.reciprocal_approx_fast(rcf[:], araw[:])
                    bcs = sb.tile([128, QS], F32, tag="bcs", bufs=2,
                                  name=f"bcs{b}_{h}_{qs_i}")
                    nc.gpsimd.partition_broadcast(bcs[:], rcf[0:1, :])
                    nc.vector.tensor_mul(at_t[64:128, q0:q0 + QS],
                                         araw[64:128, :], bcs[64:128, :])

                emit_p1_tb(0, 0)
                emit_p1_tb(0, 1)
                emit_p1_tb(0, 2)
                emit_p1_tb(0, 3)
                emit_vext(0)
                for b in range(B):
                    for h in range(2):
                        for qs_i in range(NQS):
                            emit_p2_stripe(b, h, qs_i)
                            if b + 1 < B:
                                idx = h * NQS + qs_i
                                emit_p1_tb(b + 1, idx)
                                if idx == 3:
                                    emit_vext(b + 1)
                        nc.sync.dma_start(agin[(b, h)][:],
                                          at_tiles[(b, h)][64:128, :])
                        nc.gpsimd.collective_compute(
                            "AllGather", mybir.AluOpType.bypass,
                            replica_groups=[list(range(N_CORES))],
                            ins=[agin[(b, h)][:]], outs=[agout[(b, h)][:]],
                        )

            # ---------------- P3: out-projection -------------------------
            with tc.tile_pool(name="ps3", bufs=1, space="PSUM") as ps3:
                for tb in range(NTB):
                    b = tb // 4
                    c0 = (tb % 4) * 512
                    po = ps3.tile([128, 512], F32, tag="outp", bufs=2,
                                  name=f"po{tb}")
                    for kc in range(8):
                        ast = sb.tile([128, 512], BF16, tag="ast", bufs=10,
                                      name=f"ast{tb}_{kc}")
                        src = agout[(b, kc // 4)]
                        r0 = (kc % 4) * 128
                        nc.sync.dma_start(ast[:],
                                          src[r0:r0 + 128, c0:c0 + 512])
                        nc.tensor.matmul(po[:],
                                         wo_b[:, kc * 128:(kc + 1) * 128],
                                         ast[:], start=(kc == 0),
                                         stop=(kc == 7))
                    ot = sb.tile([128, 512], F32, tag="ot", bufs=3,
                                 name=f"ot{tb}")
                    nc.vector.tensor_scalar_add(ot[:], po[:], bias_t["bo"][:])
                    nc.sync.dma_start(outT[:, tb * 512:(tb + 1) * 512], ot[:])

    nc.compile()
    return nc


def _get_nc():
    if "nc" not in _CACHE:
        _CACHE["nc"] = _build()
    return _CACHE["nc"]


def _make_in_maps(x, Wq, bq, Wk, bk, Wv, bv, Wo, bo):
    x = np.asarray(x, dtype=np.float32)
    Wq, Wk, Wv, Wo = (np.asarray(w, dtype=np.float32) for w in (Wq, Wk, Wv, Wo))
    bq, bk, bv, bo = (np.asarray(v, dtype=np.float32) for v in (bq, bk, bv, bo))

    xT = np.ascontiguousarray(x.reshape(TOK, D).T)
    # Wo rows permuted: gathered A'^T row r*64+t of head-block h corresponds
    # to head (2r+h), dim t -> original Wo row r*128 + h*64 + t.
    wo4 = Wo.reshape(N_CORES, 2, HD, D)
    wo_perm = np.concatenate([wo4[:, 0], wo4[:, 1]], axis=0).reshape(D, D)

    in_maps = []
    for c in range(N_CORES):
        cs = slice(c * CW, (c + 1) * CW)
        in_maps.append({
            "xT": xT,
            "wq": np.ascontiguousarray(Wq[:, cs]),
            "wk": np.ascontiguousarray(Wk[:, cs]),
            "wv": np.ascontiguousarray(Wv[:, cs]),
            "wo": np.ascontiguousarray(wo_perm[:, cs]),
            "bq": np.ascontiguousarray(bq[cs].reshape(CW, 1)),
            "bk": np.ascontiguousarray(bk[cs].reshape(CW, 1)),
            "bv": np.ascontiguousarray(bv[cs].reshape(CW, 1)),
            "bo": np.ascontiguousarray(bo[cs].reshape(CW, 1)),
        })
    return in_maps


def kernel(x, Wq, bq, Wk, bk, Wv, bv, Wo, bo):
    from concourse import bass_utils

    in_maps = _make_in_maps(x, Wq, bq, Wk, bk, Wv, bv, Wo, bo)
    nc = _get_nc()
    res = bass_utils.run_bass_kernel_spmd(nc, in_maps,
                                          core_ids=list(range(N_CORES)))
    _CACHE["last_results"] = res

    out = np.empty((TOK, D), dtype=np.float32)
    for c in range(N_CORES):
        out[:, c * CW:(c + 1) * CW] = res.results[c]["outT"].T
    return out.reshape(B, S, D)
